# revision 1
# baseline (speedup 1.0000x reference)
"""Trainium2 Bass kernel for AttnPainterOil-style top-K stroke compositing.

Problem semantics (per pixel, fully independent):
  draw[n] = (n+1) * (alpha[n] > 0.1); top-K=10 of draw over N=256 strokes
  (descending) == the 10 highest-index strokes with alpha > 0.1 (for the
  target input distribution every pixel has >= 10 passing strokes, checked
  on the host below).  Gather alpha/color at those indices and composite
  back-to-front over a white canvas.

Streaming formulation used on device (front-to-back, strokes in descending
index order): maintain per-pixel transmittance T (init 1), accepted count k
(init 0) and color accumulator C (init 0).  For each stroke:
  ae = a * 1{a > 0.1} * 1{k < 10}
  k += 1{ae > 0}
  ta = ae * T ;  T -= ta ;  C += ta * c
Final canvas = C + T (white background).

Only the top D=30 strokes can ever enter any pixel's top-10 (the host
verifies >= 10 passing within the top D per pixel before using the device
path; anything else falls back to an exact host replication).  This cuts
device traffic 8.5x versus streaming all 256 strokes.

Sharding: pure data parallel, one batch element per NeuronCore (B=8, 8
cores).  Engine split: the whole serial per-stroke chain and the color
products run on DVE (GpSimd shares SBUF ports with DVE and co-running
them degrades both); PE accumulates the weighted colors into PSUM via
identity matmuls; a final DVE op adds the white background straight out
of PSUM.
"""

import numpy as np

B, N, W, K = 8, 256, 128, 10
ALPHA_THRESH = 0.1
D = 30          # strokes processed from the top (must cover every pixel's top-10)
P = 128         # partitions (pixel rows)
F = 128         # free dim (pixel cols)
G = 8           # strokes per color-DMA chunk
NCORES = 8

_nc_cache = {}


def _build_nc(depth):
    import concourse.bass as bass  # noqa: F401
    import concourse.tile as tile
    from concourse import bacc, mybir
    from concourse.vector_clock import ScopedClock

    op = mybir.AluOpType
    f32 = mybir.dt.float32

    class _OneShotTileContext(tile.TileContext):
        """TileContext with a slim exit: the drain alone (it waits on the
        global clock, including output-DMA completion) — no all-engine
        barriers and no per-semaphore clears.  Safe because every
        run_bass_kernel_spmd call builds and loads a fresh executable, so
        semaphore state never carries across runs."""

        def _drain_and_barrier(self, tick_clock, wait_clock):
            drain_inst = self.nc.sync.drain()
            wait_clock.add_sem_waits(
                drain_inst.ins, ScopedClock({None: tick_clock.global_clock})
            )
            popped = self.nc._tile_sem_poison_stack.pop()
            assert popped is self._sem_poison

    nc = bacc.Bacc("TRN2", target_bir_lowering=False, debug=False)

    alpha_d = nc.dram_tensor("alpha_in", [P, depth * F], f32, kind="ExternalInput").ap()
    color_d = nc.dram_tensor("color_in", [P, depth * 3 * F], f32, kind="ExternalInput").ap()
    ident_d = nc.dram_tensor("ident_in", [P, P], f32, kind="ExternalInput").ap()
    out_d = nc.dram_tensor("out", [P, 3 * F], f32, kind="ExternalOutput").ap()

    with _OneShotTileContext(nc) as tc:
        with (
            tc.tile_pool(name="const", bufs=1) as constp,
            tc.tile_pool(name="state", bufs=1) as statep,
            tc.tile_pool(name="alpha", bufs=2) as alphap,
            tc.tile_pool(name="ae0", bufs=2) as ae0p,
            tc.tile_pool(name="cpair", bufs=4) as cpairp,
            tc.tile_pool(name="cchunk", bufs=2) as cchunkp,
            tc.tile_pool(name="tap", bufs=2) as tap,
            tc.tile_pool(name="aep", bufs=2) as aep,
            tc.tile_pool(name="prodp", bufs=4) as prodp,
            tc.tile_pool(name="psum", bufs=1, space="PSUM") as psump,
        ):
            # ident via SWDGE (gpsimd queue) so it doesn't delay the HWDGE
            # input stream; it's only needed by the first matmul.
            ident = constp.tile([P, P], f32)
            nc.gpsimd.dma_start(ident[:], ident_d)

            kcnt = statep.tile([P, F], f32)
            T = statep.tile([P, F], f32)
            nc.vector.memset(kcnt[:], 0.0)
            nc.gpsimd.memset(T[:], 1.0)

            cacc = psump.tile([P, 3 * F], f32)

            # small first chunk: the opening compute waits on 128KB of
            # alpha instead of 256KB, and the first ae0 op is 2x shorter
            sizes = [4] + [G] * ((depth - 4) // G)
            rem = depth - sum(sizes)
            if rem:
                sizes.append(rem)
            chunks = []
            off = 0
            for g_sz in sizes:
                chunks.append((off, g_sz))
                off += g_sz

            def chain_ops(ss, ae0_s, ta_out):
                """Serial per-stroke mask/count/transmittance ops (all DVE)."""
                if ss < K:
                    ae = ae0_s          # gate reads k_{ss-1} <= ss <= 9 < 10: always open
                else:
                    ae_t = aep.tile([P, F], f32, tag="ae")
                    nc.vector.scalar_tensor_tensor(
                        ae_t[:], kcnt[:], 9.5, ae0_s, op0=op.is_lt, op1=op.mult
                    )
                    ae = ae_t[:]
                if ss < depth - 1:
                    nc.vector.scalar_tensor_tensor(
                        kcnt[:], ae, 0.0, kcnt[:], op0=op.is_gt, op1=op.add
                    )
                nc.vector.tensor_tensor(ta_out, ae, T[:], op=op.mult)
                nc.vector.tensor_tensor(T[:], T[:], ta_out, op=op.subtract)

            # Everything on DVE: GpSimd shares SBUF ports with DVE and
            # co-running them degrades DVE ~5x.  PE (own xbus ports)
            # accumulates the weighted colors without contention.
            for off, g_sz in chunks:
                # alpha per chunk: first compute waits on 256KB, not the
                # whole slab
                atile = alphap.tile([P, G * F], f32, tag="alpha")
                nc.sync.dma_start(
                    atile[:, : g_sz * F], alpha_d[:, off * F : (off + g_sz) * F]
                )

                # chunk 0: color in stroke-pair slices so the first product
                # isn't gated on a big transfer; later chunks: one DMA each
                first = off == 0
                if first:
                    ctiles = []
                    for s2 in range(g_sz // 2):
                        cpair = cpairp.tile([P, 2, 3, F], f32, tag="cpair")
                        lo = (off + 2 * s2) * 3 * F
                        c_src = color_d[:, lo : lo + 2 * 3 * F]
                        nc.sync.dma_start(
                            cpair[:], c_src.rearrange("p (s c f) -> p s c f", s=2, c=3)
                        )
                        ctiles.append(cpair)
                else:
                    cchunk = cchunkp.tile([P, G, 3, F], f32, tag="cchunk")
                    lo = off * 3 * F
                    c_src = color_d[:, lo : lo + g_sz * 3 * F]
                    nc.sync.dma_start(
                        cchunk[:, :g_sz],
                        c_src.rearrange("p (s c f) -> p s c f", s=g_sz, c=3),
                    )

                # ae0 = a * 1{a > thresh} for the whole chunk (batched)
                ae0 = ae0p.tile([P, G * F], f32, tag="ae0")
                a_sl = atile[:, : g_sz * F]
                nc.vector.scalar_tensor_tensor(
                    ae0[:, : g_sz * F], a_sl, ALPHA_THRESH, a_sl,
                    op0=op.is_gt, op1=op.mult,
                )

                # stroke pairs throughout: keeps PE uniformly busy (quads
                # idle PE between bursts and trigger HAM downclock)
                bs = 2
                s = 0
                while s < g_sz:
                    b = min(bs, g_sz - s)
                    ta_grp = tap.tile([P, 2, F], f32, tag="ta")
                    for j in range(b):
                        chain_ops(off + s + j, ae0[:, (s + j) * F : (s + j + 1) * F],
                                  ta_grp[:, j])
                    prod = prodp.tile([P, 2, 3, F], f32, tag="prod")
                    if first:
                        c_grp = ctiles[s // 2][:]
                    else:
                        c_grp = cchunk[:, s : s + b]
                    ta_b = ta_grp[:, :b].unsqueeze(2).broadcast_to((P, b, 3, F))
                    nc.vector.tensor_tensor(prod[:, :b], c_grp, ta_b, op=op.mult)
                    if off + s == depth - 2:
                        # final pair: accumulate on DVE in SBUF so the PSUM
                        # matmul group closes early and PE drains in parallel
                        tailsum = constp.tile([P, 3, F], f32, tag="tailsum")
                        nc.vector.tensor_tensor(
                            tailsum[:], prod[:, 0], prod[:, 1], op=op.add
                        )
                    else:
                        for j in range(b):
                            nc.tensor.matmul(
                                cacc[:], ident[:],
                                prod[:, j].rearrange("p c f -> p (c f)"),
                                start=(off + s + j == 0),
                                stop=(off + s + j == depth - 3),
                            )
                    s += b

            # out = C_psum + (tailsum + T): the T-fold runs while PE still
            # drains; only one op depends on the final PSUM state
            T_b = T[:].unsqueeze(1).broadcast_to((P, 3, F))
            nc.vector.tensor_tensor(tailsum[:], tailsum[:], T_b, op=op.add)
            out_t = constp.tile([P, 3, F], f32, tag="out")
            nc.vector.tensor_tensor(
                out_t[:], cacc[:].rearrange("p (c f) -> p c f", c=3), tailsum[:],
                op=op.add,
            )
            nc.sync.dma_start(out_d, out_t[:].rearrange("p c f -> p (c f)"))

    nc.compile()
    return nc


def _prep_inputs(color_stroke, alpha, depth):
    """Slice the top `depth` strokes (reversed so stroke 0 = highest index)
    and lay them out per core: alpha [P, depth*F], color [P, depth*3*F]."""
    a_r = alpha[:, N - depth :, 0][:, ::-1]          # (B, depth, P, F)
    c_r = color_stroke[:, N - depth :][:, ::-1]      # (B, depth, 3, P, F)
    ident = np.eye(P, dtype=np.float32)
    in_maps = []
    for b in range(B):
        a_core = np.ascontiguousarray(a_r[b].transpose(1, 0, 2)).reshape(P, depth * F)
        c_core = np.ascontiguousarray(c_r[b].transpose(2, 0, 1, 3)).reshape(
            P, depth * 3 * F
        )
        in_maps.append(
            {"alpha_in": a_core, "color_in": c_core, "ident_in": ident}
        )
    return in_maps


def _reference_numpy(color_stroke, alpha):
    """Exact replication of the oracle (incl. top-k tie-breaking) on host.
    Only used when the depth-cutoff precondition fails (pathological inputs)."""
    stroke_ids = np.arange(1, N + 1, dtype=np.int32).reshape(1, N, 1, 1)
    draw = stroke_ids * (alpha[:, :, 0] > ALPHA_THRESH).astype(np.int32)  # (B,N,W,W)
    draw_t = np.moveaxis(draw, 1, -1)  # (B,W,W,N)
    idx = np.argsort(-draw_t, axis=-1, kind="stable")[..., :K]  # (B,W,W,K)
    idx = np.moveaxis(idx, -1, 1)[:, :, None]  # (B,K,1,W,W)
    alpha_k = np.take_along_axis(alpha, idx, axis=1)  # (B,K,1,W,W)
    color_k = np.take_along_axis(color_stroke, idx, axis=1)  # (B,K,3,W,W)
    canvas = np.ones((B, 3, W, W), dtype=color_stroke.dtype)
    for i in range(K - 1, -1, -1):
        a = alpha_k[:, i]
        canvas = canvas * (1.0 - a) + a * color_k[:, i]
    return canvas


def kernel(color_stroke, alpha):
    color_stroke = np.asarray(color_stroke, dtype=np.float32)
    alpha = np.asarray(alpha, dtype=np.float32)
    assert color_stroke.shape == (B, N, 3, W, W), color_stroke.shape
    assert alpha.shape == (B, N, 1, W, W), alpha.shape

    # Precondition for the depth cutoff: every pixel finds its 10 passing
    # strokes within the top D.  (Exact fixed input needs D* = 30.)
    top_pass = (alpha[:, N - D :, 0] > ALPHA_THRESH).sum(axis=1)
    if top_pass.min() < K:
        return _reference_numpy(color_stroke, alpha)

    from concourse.bass_utils import run_bass_kernel_spmd

    if D not in _nc_cache:
        _nc_cache[D] = _build_nc(D)
    nc = _nc_cache[D]

    in_maps = _prep_inputs(color_stroke, alpha, D)
    res = run_bass_kernel_spmd(nc, in_maps, core_ids=list(range(NCORES)))

    out = np.empty((B, 3, W, W), dtype=np.float32)
    for b in range(B):
        out[b] = res.results[b]["out"].reshape(P, 3, F).transpose(1, 0, 2)
    return out



# revision 6
# speedup vs baseline: 1.3132x; 1.3132x over previous
"""Trainium2 Bass kernel for AttnPainterOil-style top-K stroke compositing.

Problem semantics (per pixel, fully independent):
  draw[n] = (n+1) * (alpha[n] > 0.1); top-K=10 of draw over N=256 strokes
  (descending) == the 10 highest-index strokes with alpha > 0.1 (for the
  target input distribution every pixel has >= 10 passing strokes, checked
  on the host below).  Gather alpha/color at those indices and composite
  back-to-front over a white canvas.

Streaming formulation used on device (front-to-back, strokes in descending
index order): maintain per-pixel transmittance T (init 1), qualifying-count
cnt (init 0) and color accumulator C (init 0).  For each stroke:
  g   = 1{cnt_before < 10}            (gate; first 10 qualifying win)
  ae  = a * 1{a > 0.1} * g
  cnt += 1{a > 0.1}
  ta  = ae * T ;  T -= ta ;  C += ta * c
Final canvas = C + T (white background).

Only the top D=30 strokes can ever enter any pixel's top-10 (the host
verifies >= 10 passing within the top D per pixel before using the device
path; anything else falls back to an exact host replication).

v2 engine split (vs the all-DVE/f32 v1 at ~57us):
  * fp16 end to end on DVE: every tensor_tensor runs in the 2x DVE perf
    mode (measured: [128,128] fp16 tt = 134ns vs 200ns f32; stt is always
    1x, so stt ops are eliminated entirely).
  * ae0 = a*1{a>0.1} is resolved on host in f32 (exact threshold compare)
    and shipped as fp16, halving input DMA.
  * The count/gate chain moves off DVE's 1x stt path: ACT computes
    q = Sign(ae0) per chunk and the gate g = Sigmoid(-40*cnt + 380)
    (exactly 0.0/1.0 in fp16); DVE only does 2x adds/mults.  ACT runs
    fully concurrent with DVE (measured: zero interference).
  * The cnt chain is independent of the T chain, so it is emitted a few
    strokes ahead to hide ACT->DVE latency.
  * PE accumulates the weighted colors into PSUM via fp16 identity
    matmuls; a final DVE op adds the white background out of PSUM.

Sharding: pure data parallel, one batch element per NeuronCore (B=8).
"""

import numpy as np

B, N, W, K = 8, 256, 128, 10
ALPHA_THRESH = 0.1
D = 30          # strokes processed from the top (must cover every pixel's top-10)
P = 128         # partitions (pixel rows)
F = 128         # free dim (pixel cols)
NCORES = 8

# gate = Sigmoid(GATE_SCALE*cnt + GATE_BIAS): cnt<=9 -> 1.0, cnt>=10 -> 0.0 (fp16)
GATE_SCALE = -40.0
GATE_BIAS = 9.5 * 40.0

_nc_cache = {}


def _build_nc(depth):
    import concourse.bass as bass  # noqa: F401
    import concourse.tile as tile
    from concourse import bacc, mybir
    from concourse.vector_clock import ScopedClock

    op = mybir.AluOpType
    f32 = mybir.dt.float32
    f16 = mybir.dt.float16
    actf = mybir.ActivationFunctionType

    class _OneShotTileContext(tile.TileContext):
        """TileContext with a slim exit: the drain alone (it waits on the
        global clock, including output-DMA completion) — no all-engine
        barriers and no per-semaphore clears.  Safe because every
        run_bass_kernel_spmd call builds and loads a fresh executable, so
        semaphore state never carries across runs."""

        def _drain_and_barrier(self, tick_clock, wait_clock):
            drain_inst = self.nc.sync.drain()
            wait_clock.add_sem_waits(
                drain_inst.ins, ScopedClock({None: tick_clock.global_clock})
            )
            popped = self.nc._tile_sem_poison_stack.pop()
            assert popped is self._sem_poison

    nc = bacc.Bacc("TRN2", target_bir_lowering=False, debug=False)

    ae_d = nc.dram_tensor("ae_in", [P, depth * F], f16, kind="ExternalInput").ap()
    color_d = nc.dram_tensor("color_in", [P, depth * 3 * F], f16, kind="ExternalInput").ap()
    ident_d = nc.dram_tensor("ident_in", [P, P], f16, kind="ExternalInput").ap()
    out_d = nc.dram_tensor("out", [P, 3 * F], f16, kind="ExternalOutput").ap()

    # chunk sizes: small first chunks so the opening compute waits on a
    # small transfer; pairs never straddle a chunk boundary
    sizes = [2, 4, 8, 8, 8]
    assert sum(sizes) == depth
    chunk_of = []          # stroke -> chunk index
    chunk_start = []
    off = 0
    for ci, g_sz in enumerate(sizes):
        chunk_start.append(off)
        chunk_of.extend([ci] * g_sz)
        off += g_sz

    with _OneShotTileContext(nc) as tc:
        with (
            tc.tile_pool(name="const", bufs=1) as constp,
            tc.tile_pool(name="state", bufs=1) as statep,
            tc.tile_pool(name="ae", bufs=2) as aep,
            tc.tile_pool(name="q", bufs=2) as qp,
            tc.tile_pool(name="cnt", bufs=3) as cntp,
            tc.tile_pool(name="gate", bufs=3) as gatep,
            tc.tile_pool(name="aeg", bufs=2) as aegp,
            tc.tile_pool(name="cpair", bufs=4) as cpairp,
            tc.tile_pool(name="cchunk", bufs=2) as cchunkp,
            tc.tile_pool(name="tap", bufs=2) as tap,
            tc.tile_pool(name="prodp", bufs=4) as prodp,
            tc.tile_pool(name="psum", bufs=1, space="PSUM") as psump,
        ):
            # ident via SWDGE (gpsimd queue) so it doesn't delay the HWDGE
            # input stream; it's only needed by the first matmul.
            ident = constp.tile([P, P], f16)
            nc.gpsimd.dma_start(ident[:], ident_d)

            T = statep.tile([P, F], f16)
            cnt0 = statep.tile([P, F], f16)
            warm = statep.tile([P, 1], f16)
            gbias = statep.tile([P, 1], f32)
            nc.vector.memset(T[:], 1.0)
            nc.vector.memset(cnt0[:], 0.0)
            nc.gpsimd.memset(warm[:], 0.0)
            nc.gpsimd.memset(gbias[:], GATE_BIAS)
            # force the ACT function-table load at t~0 (it otherwise stalls
            # the first real ACT op by ~1.3us)
            nc.scalar.sign(warm[:], warm[:])

            cacc = psump.tile([P, 3 * F], f32)

            ae_tiles = {}       # chunk -> (tile, size)
            q_tiles = {}
            c_tiles = {}        # chunk -> tile ([P, g, 3, F]) or list of pair tiles

            def emit_chunk_io(ci):
                g_sz = sizes[ci]
                o = chunk_start[ci]
                atile = aep.tile([P, 8 * F], f16, tag="ae")
                nc.sync.dma_start(
                    atile[:, : g_sz * F], ae_d[:, o * F : (o + g_sz) * F]
                )
                ae_tiles[ci] = atile
                qtile = qp.tile([P, 8 * F], f16, tag="q")
                nc.scalar.sign(qtile[:, : g_sz * F], atile[:, : g_sz * F])
                q_tiles[ci] = qtile
                if ci <= 1:
                    # opening chunks: color in stroke-pair slices so the first
                    # product isn't gated on a big transfer
                    pairs = []
                    for s2 in range(g_sz // 2):
                        cpair = cpairp.tile([P, 2, 3, F], f16, tag="cpair")
                        lo = (o + 2 * s2) * 3 * F
                        c_src = color_d[:, lo : lo + 2 * 3 * F]
                        nc.sync.dma_start(
                            cpair[:], c_src.rearrange("p (s c f) -> p s c f", s=2, c=3)
                        )
                        pairs.append(cpair)
                    c_tiles[ci] = pairs
                else:
                    cchunk = cchunkp.tile([P, 8, 3, F], f16, tag="cchunk")
                    lo = o * 3 * F
                    c_src = color_d[:, lo : lo + g_sz * 3 * F]
                    nc.sync.dma_start(
                        cchunk[:, :g_sz],
                        c_src.rearrange("p (s c f) -> p s c f", s=g_sz, c=3),
                    )
                    c_tiles[ci] = cchunk

            def ae_plane(s, n=1):
                ci = chunk_of[s]
                lo = s - chunk_start[ci]
                return ae_tiles[ci][:, lo * F : (lo + n) * F]

            def q_plane(s):
                ci = chunk_of[s]
                lo = s - chunk_start[ci]
                return q_tiles[ci][:, lo * F : (lo + 1) * F]

            def c_pair(s):
                ci = chunk_of[s]
                lo = s - chunk_start[ci]
                if ci <= 1:
                    return c_tiles[ci][lo // 2][:]
                return c_tiles[ci][:, lo : lo + 2]

            emit_chunk_io(0)
            emit_chunk_io(1)
            next_chunk = 2

            # cnt_t tiles (count of qualifying strokes 0..t); gates per pair
            cnt_tiles = {-1: cnt0}
            gate_tiles = {}     # pair start stroke (>=10, even) -> [P,2,F] tile
            cnt_done = -1

            def emit_cnt_upto(target):
                nonlocal cnt_done
                target = min(target, depth - 2)   # cnt_28 is the last needed
                while cnt_done < target:
                    t = cnt_done + 1
                    if chunk_of[t] not in q_tiles:
                        break
                    ct = cntp.tile([P, F], f16, tag="cnt")
                    nc.vector.tensor_tensor(
                        ct[:], cnt_tiles[t - 1][:], q_plane(t), op=op.add
                    )
                    cnt_tiles[t] = ct
                    cnt_done = t
                    # gate for stroke t+1 = f(cnt_t), batched into pair tiles
                    u = t + 1
                    if u >= K and u <= depth - 1:
                        ps = u - ((u - K) % 2)   # even pair start
                        if ps not in gate_tiles:
                            gtile = gatep.tile([P, 2, F], f16, tag="gate")
                            gate_tiles[ps] = gtile
                        nc.scalar.activation(
                            gate_tiles[ps][:, u - ps], cnt_tiles[t][:],
                            func=actf.Sigmoid, bias=gbias[:], scale=GATE_SCALE,
                        )
                    # free old cnt tiles implicitly via pool rotation

            for s in range(0, depth, 2):
                if next_chunk < len(sizes) and s >= chunk_start[next_chunk - 1]:
                    emit_chunk_io(next_chunk)
                    next_chunk += 1
                emit_cnt_upto(s + 3)

                ta_grp = tap.tile([P, 2, F], f16, tag="ta")
                if s < K:
                    # gate always open for the first 10 strokes
                    for j in range(2):
                        nc.vector.tensor_tensor(
                            ta_grp[:, j], ae_plane(s + j), T[:], op=op.mult
                        )
                        nc.vector.tensor_tensor(T[:], T[:], ta_grp[:, j], op=op.subtract)
                else:
                    aeg = aegp.tile([P, 2, F], f16, tag="aeg")
                    nc.vector.tensor_tensor(
                        aeg[:].rearrange("p s f -> p (s f)"), ae_plane(s, 2),
                        gate_tiles[s][:].rearrange("p s f -> p (s f)"), op=op.mult,
                    )
                    for j in range(2):
                        nc.vector.tensor_tensor(
                            ta_grp[:, j], aeg[:, j], T[:], op=op.mult
                        )
                        nc.vector.tensor_tensor(T[:], T[:], ta_grp[:, j], op=op.subtract)

                prod = prodp.tile([P, 2, 3, F], f16, tag="prod")
                ta_b = ta_grp[:].unsqueeze(2).broadcast_to((P, 2, 3, F))
                nc.vector.tensor_tensor(prod[:], c_pair(s), ta_b, op=op.mult)
                if s == depth - 2:
                    # final pair: accumulate on DVE in SBUF so the PSUM
                    # matmul group closes early and PE drains in parallel
                    tailsum = constp.tile([P, 3, F], f16, tag="tailsum")
                    nc.vector.tensor_tensor(
                        tailsum[:], prod[:, 0], prod[:, 1], op=op.add
                    )
                else:
                    for j in range(2):
                        nc.tensor.matmul(
                            cacc[:], ident[:],
                            prod[:, j].rearrange("p c f -> p (c f)"),
                            start=(s + j == 0),
                            stop=(s + j == depth - 3),
                        )

            # out = C_psum + (tailsum + T): the T-fold runs while PE still
            # drains; only one op depends on the final PSUM state
            T_b = T[:].unsqueeze(1).broadcast_to((P, 3, F))
            nc.vector.tensor_tensor(tailsum[:], tailsum[:], T_b, op=op.add)
            out_t = constp.tile([P, 3, F], f16, tag="out")
            nc.vector.tensor_tensor(
                out_t[:], cacc[:].rearrange("p (c f) -> p c f", c=3), tailsum[:],
                op=op.add,
            )
            nc.sync.dma_start(out_d, out_t[:].rearrange("p c f -> p (c f)"))

    nc.compile()
    return nc


def _prep_inputs(color_stroke, alpha, depth):
    """Slice the top `depth` strokes (reversed so stroke 0 = highest index),
    resolve the alpha threshold in f32 on host, and lay out per core in fp16:
    ae [P, depth*F], color [P, depth*3*F]."""
    a_r = alpha[:, N - depth :, 0][:, ::-1]          # (B, depth, P, F) f32
    ae0 = (a_r * (a_r > ALPHA_THRESH)).astype(np.float16)
    c_r = color_stroke[:, N - depth :][:, ::-1].astype(np.float16)  # (B, depth, 3, P, F)
    ident = np.eye(P, dtype=np.float16)
    in_maps = []
    for b in range(B):
        a_core = np.ascontiguousarray(ae0[b].transpose(1, 0, 2)).reshape(P, depth * F)
        c_core = np.ascontiguousarray(c_r[b].transpose(2, 0, 1, 3)).reshape(
            P, depth * 3 * F
        )
        in_maps.append(
            {"ae_in": a_core, "color_in": c_core, "ident_in": ident}
        )
    return in_maps


def _reference_numpy(color_stroke, alpha):
    """Exact replication of the oracle (incl. top-k tie-breaking) on host.
    Only used when the depth-cutoff precondition fails (pathological inputs)."""
    stroke_ids = np.arange(1, N + 1, dtype=np.int32).reshape(1, N, 1, 1)
    draw = stroke_ids * (alpha[:, :, 0] > ALPHA_THRESH).astype(np.int32)  # (B,N,W,W)
    draw_t = np.moveaxis(draw, 1, -1)  # (B,W,W,N)
    idx = np.argsort(-draw_t, axis=-1, kind="stable")[..., :K]  # (B,W,W,K)
    idx = np.moveaxis(idx, -1, 1)[:, :, None]  # (B,K,1,W,W)
    alpha_k = np.take_along_axis(alpha, idx, axis=1)  # (B,K,1,W,W)
    color_k = np.take_along_axis(color_stroke, idx, axis=1)  # (B,K,3,W,W)
    canvas = np.ones((B, 3, W, W), dtype=color_stroke.dtype)
    for i in range(K - 1, -1, -1):
        a = alpha_k[:, i]
        canvas = canvas * (1.0 - a) + a * color_k[:, i]
    return canvas


def kernel(color_stroke, alpha):
    color_stroke = np.asarray(color_stroke, dtype=np.float32)
    alpha = np.asarray(alpha, dtype=np.float32)
    assert color_stroke.shape == (B, N, 3, W, W), color_stroke.shape
    assert alpha.shape == (B, N, 1, W, W), alpha.shape

    # Precondition for the depth cutoff: every pixel finds its 10 passing
    # strokes within the top D.  (Exact fixed input needs D* = 30.)
    top_pass = (alpha[:, N - D :, 0] > ALPHA_THRESH).sum(axis=1)
    if top_pass.min() < K:
        return _reference_numpy(color_stroke, alpha)

    from concourse.bass_utils import run_bass_kernel_spmd

    if D not in _nc_cache:
        _nc_cache[D] = _build_nc(D)
    nc = _nc_cache[D]

    in_maps = _prep_inputs(color_stroke, alpha, D)
    res = run_bass_kernel_spmd(nc, in_maps, core_ids=list(range(NCORES)))

    out = np.empty((B, 3, W, W), dtype=np.float32)
    for b in range(B):
        out[b] = (
            res.results[b]["out"].astype(np.float32).reshape(P, 3, F).transpose(1, 0, 2)
        )
    return out
